# revision 15
# baseline (speedup 1.0000x reference)
"""5-layer GAT on 8 TRN2 NeuronCores — dst-sharded gather/aggregate kernel (v2).

Strategy (per core c of 8):
  - dst nodes [c*6250, (c+1)*6250) and their (dst-sorted) edges, grouped in
    128-dst windows; window edges split by src half (int16 gather idx limit),
    padded to 128-edge chunks (uniform structure across cores for SPMD).
  - L1: transpose-dma_gather of raw x rows (256B each) + per-chunk PE matmul
    z1 = x_g^T @ [W1|wl1]  (kills the replicated z1 table of v1).
  - L2-5: dma_gather [z|el] rows (bf16) from a replicated table T_l.
  - Scores: el from gather; er expanded per-edge via PE matmul (lhsT=ST mask);
    e = el + er on DVE; LeakyReLU+Exp on the Scalar engine (ACT, alpha=0.2).
  - Aggregation: psw[:, :hf] += S_k^T @ (ex*z), psw[:, hf:hf+h] += S_k^T @ ex
    (two PE matmuls per chunk; no strided DVE appends).
  - Window flush: divide by sum, +bias, elu via ACT Relu/Exp composition,
    transpose h to [feat, node] on PE for the next layer's z matmul.
  - z_{l+1} = h_shard @ [W|wl|wr] per shard; AllGather the packed table.
No tensor_scalar ops (slow path on this DVE); constants via broadcast APs.
"""
import numpy as np

N = 50000
E = 800000
NC = 8
SH = N // NC
WIN = 128
NWIN = (SH + WIN - 1) // WIN
HALF = 25000
LAYERS = [(128, 64, 4), (256, 64, 2), (128, 64, 2), (128, 64, 1), (64, 4, 1)]
NEG = 0.2
MAXCH = 8


def _row_units(hf, h):
    u = hf + 2 * h
    return ((u + 127) // 128) * 128


ROWS_U = [_row_units(h * f, h) for (_, f, h) in LAYERS]


def _prep(src, dst):
    order = np.argsort(dst, kind="stable")
    src_s, dst_s = src[order], dst[order]
    core_of = dst_s // SH
    core_lists = []
    nch = np.zeros((NWIN, 2), np.int64)
    for c in range(NC):
        m = core_of == c
        s, d = src_s[m], dst_s[m] - c * SH
        w = d // WIN
        lists = {}
        for wi in range(NWIN):
            mw = w == wi
            sw, dw = s[mw], d[mw]
            lo = sw < HALF
            lists[(wi, 0)] = (sw[lo].astype(np.int64), dw[lo])
            lists[(wi, 1)] = (sw[~lo].astype(np.int64) - HALF, dw[~lo])
            for hf in range(2):
                nch[wi, hf] = max(nch[wi, hf],
                                  (len(lists[(wi, hf)][0]) + 127) // 128)
        core_lists.append(lists)
    nch = np.maximum(nch, 1)

    calls = []            # (win, half, c0, g)
    win_first = {}
    win_last = {}
    c0 = 0
    for wi in range(NWIN):
        win_first[wi] = c0
        for hf in range(2):
            n = int(nch[wi, hf])
            k = 0
            while k < n:
                g = min(MAXCH, n - k)
                calls.append((wi, hf, c0 + k, g))
                k += g
            c0 += n
        win_last[wi] = c0 - 1
    NCH = c0
    EPAD = NCH * 128

    idx_streams, ldst_cols, ldst_rows = [], [], []
    for c in range(NC):
        lists = core_lists[c]
        idx = np.zeros(EPAD, np.int64)
        ld = np.full(EPAD, -1, np.int64)
        pos = 0
        for wi in range(NWIN):
            for hf in range(2):
                s, d = lists[(wi, hf)]
                n = int(nch[wi, hf]) * 128
                idx[pos:pos + len(s)] = s
                ld[pos:pos + len(d)] = d % WIN
                pos += n
        blocks = []
        for (wi, hf, cc0, g) in calls:
            blk = idx[cc0 * 128:(cc0 + g) * 128].astype(np.int16)
            blocks.append(np.tile(blk.reshape(-1, 16).T, (8, 1)))
        idx_streams.append(np.ascontiguousarray(np.concatenate(blocks, axis=1)))
        ldst_cols.append(np.ascontiguousarray(
            ld.reshape(NCH, 128).T.astype(np.int8)))
        rows = np.zeros((len(calls), 1, MAXCH * 128), np.int8)
        for i, (wi, hf, cc0, g) in enumerate(calls):
            rows[i, 0, :g * 128] = ld[cc0 * 128:(cc0 + g) * 128]
        ldst_rows.append(rows)

    return calls, win_first, win_last, NCH, idx_streams, ldst_cols, ldst_rows


def _build(calls, win_first, win_last, NCH, only=None):
    from contextlib import ExitStack
    import concourse.bass as bass
    import concourse.bacc as bacc
    import concourse.tile as tile
    from concourse import mybir
    from concourse.masks import make_identity

    F32, BF16, I16 = mybir.dt.float32, mybir.dt.bfloat16, mybir.dt.int16
    I8 = mybir.dt.int8
    EXP = mybir.ActivationFunctionType.Exp
    CPY = mybir.ActivationFunctionType.Copy
    RELU = mybir.ActivationFunctionType.Relu
    EQ = mybir.AluOpType.is_equal
    MAX = mybir.AluOpType.max
    MUL = mybir.AluOpType.mult
    ADD = mybir.AluOpType.add
    TOT16 = NCH * 8
    NCALLS = len(calls)

    nc = bacc.Bacc("TRN2", num_devices=NC, num_swdge_queues=4)

    xrow_d = nc.dram_tensor("xrows", [N, 128], BF16, kind="ExternalInput")
    xTs = nc.dram_tensor("xTs", [128, SH], BF16, kind="ExternalInput")
    Waug, btens = [], []
    for li, (fin, fo, h) in enumerate(LAYERS, 1):
        hf = h * fo
        Waug.append(nc.dram_tensor(f"Waug{li}", [fin, hf + 2 * h], BF16,
                                   kind="ExternalInput"))
        btens.append(nc.dram_tensor(f"bb{li}", [1, hf], F32, kind="ExternalInput"))
    idxs_d = nc.dram_tensor("idxs", [128, TOT16], I16, kind="ExternalInput")
    ldstc_d = nc.dram_tensor("ldstc", [128, NCH], I8, kind="ExternalInput")
    ldstr_d = nc.dram_tensor("ldstr", [NCALLS, 1, MAXCH * 128], I8,
                             kind="ExternalInput")
    out_d = nc.dram_tensor("out", [SH, 4], F32, kind="ExternalOutput")

    T = [None]  # layer-1 gathers straight from xrows
    cc_in = [None]
    for li in range(2, 6):
        u = 128 if li == 3 else ROWS_U[li - 1]   # L3 gathers raw h2 rows
        cc_in.append(nc.dram_tensor(f"ccin{li}", [SH, u], BF16, kind="Internal"))
        T.append(nc.dram_tensor(f"T{li}", [N, u], BF16, kind="Internal",
                                addr_space="Shared"))
    rg = [list(range(NC))]

    def bcast_last(ap, n):
        """free-broadcast an AP whose last dim is [*, 1] to [0, n]."""
        a = [list(p) for p in ap.ap]
        assert a[-1][1] == 1
        a[-1] = [0, n]
        return bass.AP(tensor=ap.tensor, offset=ap.offset, ap=a)

    def ap3(base, dims):
        """3D AP on a 2D slice: dims = [[s1,n1],[s2,n2]] appended to partition dim."""
        return bass.AP(tensor=base.tensor, offset=base.offset,
                       ap=[list(base.ap[0])] + [list(d) for d in dims])

    with tile.TileContext(nc) as tc:
        with ExitStack() as ctx:
            cpool = ctx.enter_context(tc.tile_pool(name="const", bufs=1))
            gpool = ctx.enter_context(tc.tile_pool(name="gat", bufs=10))
            xgpool = ctx.enter_context(tc.tile_pool(name="xg", bufs=4))
            spool = ctx.enter_context(tc.tile_pool(name="masks", bufs=6))
            rpool = ctx.enter_context(tc.tile_pool(name="rhs", bufs=4))
            zxpool = ctx.enter_context(tc.tile_pool(name="zx", bufs=3))
            epool = ctx.enter_context(tc.tile_pool(name="expx", bufs=8))
            lpool = ctx.enter_context(tc.tile_pool(name="ldr", bufs=6))
            wpool = ctx.enter_context(tc.tile_pool(name="wflush", bufs=4))
            zpool = ctx.enter_context(tc.tile_pool(name="zphase", bufs=3))
            pp_w = ctx.enter_context(tc.tile_pool(name="ps_w", bufs=2, space="PSUM"))
            pp_er = ctx.enter_context(tc.tile_pool(name="ps_er", bufs=2, space="PSUM"))
            pp_z = ctx.enter_context(tc.tile_pool(name="ps_z", bufs=2, space="PSUM"))
            pp_s = ctx.enter_context(tc.tile_pool(name="ps_s", bufs=2, space="PSUM"))

            iota_row = cpool.tile([128, 128], I8)
            nc.gpsimd.iota(iota_row[:, :], pattern=[[1, 128]], base=0,
                           channel_multiplier=0,
                           allow_small_or_imprecise_dtypes=True)
            iota_col = cpool.tile([128, 128], I8)
            nc.gpsimd.iota(iota_col[:, :], pattern=[[0, 128]], base=0,
                           channel_multiplier=1,
                           allow_small_or_imprecise_dtypes=True)
            ident = cpool.tile([128, 128], BF16)
            make_identity(nc, ident[:, :])
            konst = cpool.tile([128, 2], F32)
            nc.vector.memset(konst[:, 0:1], 1e-30)
            nc.vector.memset(konst[:, 1:2], -1.0)

            idx_sb = cpool.tile([128, TOT16], I16)
            nc.sync.dma_start(out=idx_sb[:, :], in_=idxs_d[:, :])
            ldstc_sb = cpool.tile([128, NCH], I8)
            nc.sync.dma_start(out=ldstc_sb[:, :], in_=ldstc_d[:, :])

            wsb, bsb = [], []
            for li, (fin, fo, h) in enumerate(LAYERS, 1):
                hf = h * fo
                cols = hf + 2 * h
                kch = (fin + 127) // 128
                wt = cpool.tile([128, kch, cols], BF16, tag=f"w{li}")
                if kch > 1:
                    nc.sync.dma_start(
                        out=wt[:, :, :],
                        in_=Waug[li - 1][:, :].rearrange("(k p) c -> p k c", p=128))
                else:
                    nc.sync.dma_start(out=wt[:fin, 0, :], in_=Waug[li - 1][:, :])
                wsb.append(wt)
                bt = cpool.tile([128, hf], F32, tag=f"b{li}")
                bsrc = btens[li - 1][:, :]
                nc.sync.dma_start(out=bt[:, :], in_=bass.AP(
                    tensor=bsrc.tensor, offset=bsrc.offset,
                    ap=[[0, 128]] + [list(p) for p in bsrc.ap[1:]]))
                bsb.append(bt)

            er_sh = cpool.tile([128, NWIN, 4], BF16)
            hT = [cpool.tile([128, SH], BF16, tag=f"hT{k}", name=f"hT{k}")
                  for k in range(2)]

            # ---------- L1 er for own shard (from xTs) ----------
            fin, fo, h = LAYERS[0]
            hf = h * fo
            xts_sb = cpool.tile([128, SH], BF16)
            nc.sync.dma_start(out=xts_sb[:, :], in_=xTs[:, :])
            for wi in range(NWIN):
                m = min(WIN, SH - wi * WIN)
                ps = pp_er.tile([128, MAXCH * 4], F32, tag="erps")
                nc.tensor.matmul(ps[:m, :h], lhsT=xts_sb[:, wi * WIN:wi * WIN + m],
                                 rhs=wsb[0][:, 0, hf + h:hf + 2 * h],
                                 start=True, stop=True)
                nc.vector.tensor_copy(er_sh[:m, wi, :h], ps[:m, :h])

            # ---------- layers ----------
            for li, (fin, fo, h) in enumerate(LAYERS, 1):
                if only == "l1" and li > 1:
                    break
                hf = h * fo
                ru = ROWS_U[li - 1]
                psw = None
                off16 = 0
                if only == "l25" and li == 1:
                    continue
                for ci, (wi, half, c0, g) in enumerate(calls):
                    ni = g * 128
                    if c0 == win_first[wi]:
                        psw = pp_w.tile([128, hf + h], F32, tag="psw")
                        psums = pp_s.tile([128, 4], F32, tag="psums")

                    if li in (1, 3):
                        ftbl = xrow_d if li == 1 else T[2]
                        base = (ftbl[0:HALF, :] if half == 0
                                else ftbl[HALF:N, :])
                        xg = xgpool.tile([128, 1, MAXCH * 128], BF16, tag="xg")
                        # transpose-mode SWDGE gathers wedge beyond 512 idxs
                        for s0 in range(0, ni, 512):
                            nn = min(512, ni - s0)
                            nc.gpsimd.dma_gather(
                                xg[:, :, s0:s0 + nn], base,
                                idx_sb[:, off16 + s0 // 16:
                                       off16 + (s0 + nn) // 16],
                                num_idxs=nn, num_idxs_reg=nn, elem_size=128,
                                transpose=True,
                                queue_num=(ci + s0 // 512) % 4)
                    else:
                        tbl = T[li - 1]
                        base = tbl[0:HALF, :] if half == 0 else tbl[HALF:N, :]
                        g_t = gpool.tile([128, MAXCH, ru], BF16, tag="gt")
                        nc.gpsimd.dma_gather(
                            g_t[:, :g, :], base, idx_sb[:, off16:off16 + g * 8],
                            num_idxs=ni, num_idxs_reg=ni, elem_size=ru,
                            queue_num=ci % 4)
                    off16 += g * 8

                    ldr = lpool.tile([128, MAXCH * 128], I8)
                    lsrc = ldstr_d[ci, :, :g * 128]
                    nc.sync.dma_start(out=ldr[:, :g * 128], in_=bass.AP(
                        tensor=lsrc.tensor, offset=lsrc.offset,
                        ap=[[0, 128]] + [list(p) for p in lsrc.ap[1:]]))

                    S_b = spool.tile([128, MAXCH, 128], BF16, tag="S")
                    in0 = bass.AP(tensor=ldstc_sb[:, :].tensor,
                                  offset=ldstc_sb[:, :].offset + c0,
                                  ap=[[NCH, 128], [1, g], [0, 128]])
                    in1 = bass.AP(tensor=iota_row[:, :].tensor,
                                  offset=iota_row[:, :].offset,
                                  ap=[[128, 128], [0, g], [1, 128]])
                    nc.vector.tensor_tensor(out=S_b[:, :g, :], in0=in0, in1=in1, op=EQ)

                    ST_b = spool.tile([128, MAXCH, 128], BF16, tag="ST")
                    in0 = bass.AP(tensor=iota_col[:, :].tensor,
                                  offset=iota_col[:, :].offset,
                                  ap=[[128, 128], [0, g], [1, 128]])
                    in1 = bass.AP(tensor=ldr[:, :].tensor,
                                  offset=ldr[:, :].offset,
                                  ap=[[MAXCH * 128, 128], [128, g], [1, 128]])
                    nc.vector.tensor_tensor(out=ST_b[:, :g, :], in0=in0, in1=in1, op=EQ)

                    er_ps = pp_er.tile([128, MAXCH * 4], F32, tag="erps")
                    for k in range(g):
                        nc.tensor.matmul(er_ps[:, k * h:(k + 1) * h],
                                         lhsT=ST_b[:, k, :],
                                         rhs=er_sh[:, wi, :h], start=True, stop=True)

                    e_t = epool.tile([128, MAXCH * 4], F32, tag="e")
                    erv = bass.AP(tensor=er_ps[:, :].tensor,
                                  offset=er_ps[:, :].offset,
                                  ap=[[MAXCH * 4, 128], [h, g], [1, h]])
                    if li in (1, 3):
                        # per-chunk z matmul + evacuate z/el to SBUF
                        zsrc = zxpool.tile([128, MAXCH, hf], BF16, tag="zx")
                        el_s = epool.tile([128, MAXCH * 4], F32, tag="els")
                        for k in range(g):
                            psz = pp_z.tile([128, hf + h], F32, tag="psz")
                            nc.tensor.matmul(
                                psz[:, :], lhsT=xg[:, 0, k * 128:(k + 1) * 128],
                                rhs=wsb[li - 1][:, 0, :hf + h],
                                start=True, stop=True)
                            nc.scalar.activation(zsrc[:, k, :], psz[:, :hf], CPY)
                            nc.scalar.activation(el_s[:, k * h:(k + 1) * h],
                                                 psz[:, hf:hf + h], CPY)
                        nc.vector.tensor_tensor(
                            out=bass.AP(tensor=e_t[:, :].tensor,
                                        offset=e_t[:, :].offset,
                                        ap=[[MAXCH * 4, 128], [1, g * h]]),
                            in0=el_s[:, :g * h], in1=erv, op=ADD)
                    else:
                        zsrc = g_t
                        elv = g_t[:, :g, hf:hf + 2 * h].bitcast(F32)
                        ev = bass.AP(tensor=e_t[:, :].tensor,
                                     offset=e_t[:, :].offset,
                                     ap=[[MAXCH * 4, 128], [h, g], [1, h]])
                        nc.vector.tensor_tensor(out=ev, in0=elv, in1=erv, op=ADD)
                    q1_t = epool.tile([128, MAXCH * 4], F32, tag="q1t")
                    nc.scalar.activation(q1_t[:, :g * h], e_t[:, :g * h], EXP)
                    q2_t = epool.tile([128, MAXCH * 4], F32, tag="q2t")
                    nc.scalar.activation(q2_t[:, :g * h], e_t[:, :g * h], EXP,
                                         scale=NEG)
                    exb = epool.tile([128, MAXCH * 4], BF16, tag="exb")
                    nc.vector.tensor_tensor(out=exb[:, :g * h],
                                            in0=q1_t[:, :g * h],
                                            in1=q2_t[:, :g * h], op=MAX)

                    rhs_t = rpool.tile([128, MAXCH, hf], BF16, tag="rhs")
                    for hi in range(h):
                        exv = bass.AP(tensor=exb[:, :].tensor,
                                      offset=exb[:, :].offset + hi,
                                      ap=[[MAXCH * 4, 128], [h, g], [0, fo]])
                        nc.vector.tensor_tensor(
                            out=rhs_t[:, :g, hi * fo:(hi + 1) * fo],
                            in0=zsrc[:, :g, hi * fo:(hi + 1) * fo], in1=exv, op=MUL)

                    for k in range(g):
                        kg = c0 + k
                        nc.tensor.matmul(psw[:, :hf], lhsT=S_b[:, k, :],
                                         rhs=rhs_t[:, k, :],
                                         start=(kg == win_first[wi]),
                                         stop=(kg == win_last[wi]))
                        nc.tensor.matmul(psums[:, :h], lhsT=S_b[:, k, :],
                                         rhs=exb[:, k * h:(k + 1) * h],
                                         start=(kg == win_first[wi]),
                                         stop=(kg == win_last[wi]))

                    if c0 + g - 1 == win_last[wi]:
                        # -------- window flush --------
                        m = min(WIN, SH - wi * WIN)
                        sg = wpool.tile([128, 4], F32, tag="sg")
                        nc.vector.tensor_tensor(
                            out=sg[:m, :h], in0=psums[:m, :h],
                            in1=bcast_last(konst[:m, 0:1], h), op=MAX)
                        rr = wpool.tile([128, 4], F32, tag="rr")
                        nc.vector.reciprocal(rr[:m, :h], sg[:m, :h])
                        ow = wpool.tile([128, hf], F32, tag="ow")
                        nc.vector.tensor_tensor(
                            out=ap3(ow[:m, :], [[fo, h], [1, fo]]),
                            in0=ap3(psw[:m, :], [[fo, h], [1, fo]]),
                            in1=ap3(rr[:m, :], [[1, h], [0, fo]]), op=MUL)
                        nc.vector.tensor_add(ow[:m, :], ow[:m, :], bsb[li - 1][:m, :])
                        if li == 5:
                            nc.sync.dma_start(out=out_d[wi * WIN:wi * WIN + m, :],
                                              in_=ow[:m, :4])
                        else:
                            p_t = wpool.tile([128, hf], F32, tag="p")
                            nc.scalar.activation(p_t[:m, :], ow[:m, :], RELU)
                            r_t = wpool.tile([128, hf], F32, tag="r")
                            nc.scalar.activation(r_t[:m, :], ow[:m, :], RELU,
                                                 scale=-1.0)
                            q_t = wpool.tile([128, hf], F32, tag="q")
                            nc.scalar.activation(q_t[:m, :], r_t[:m, :], EXP,
                                                 scale=-1.0)
                            s_t = wpool.tile([128, hf], F32, tag="s")
                            nc.vector.tensor_tensor(out=s_t[:m, :], in0=p_t[:m, :],
                                                    in1=q_t[:m, :], op=ADD)
                            hbf = wpool.tile([128, hf], BF16, tag="hbf")
                            nc.vector.tensor_tensor(
                                out=hbf[:m, :], in0=s_t[:m, :],
                                in1=bcast_last(konst[:m, 1:2], hf), op=ADD)
                            if li == 2:
                                nc.sync.dma_start(
                                    out=cc_in[2][wi * WIN:wi * WIN + m, :],
                                    in_=hbf[:m, :])
                            for k in range((hf + 127) // 128):
                                kk = min(128, hf - k * 128)
                                pt = pp_er.tile([128, 128], BF16, tag="erps")
                                nc.tensor.transpose(
                                    pt[:kk, :m], hbf[:m, k * 128:k * 128 + kk],
                                    ident[:m, :m])
                                nc.vector.tensor_copy(
                                    hT[k][:kk, wi * WIN:wi * WIN + m], pt[:kk, :m])

                # -------- z phase for next layer + AllGather --------
                if li == 2:
                    # h2 rows already written at flush; only er3 needed
                    fin2, fo2, h2 = LAYERS[2]
                    hf2 = h2 * fo2
                    for wi in range(NWIN):
                        m = min(WIN, SH - wi * WIN)
                        ps = pp_er.tile([128, MAXCH * 4], F32, tag="erps")
                        nc.tensor.matmul(
                            ps[:m, :h2], lhsT=hT[0][:, wi * WIN:wi * WIN + m],
                            rhs=wsb[2][:, 0, hf2 + h2:hf2 + 2 * h2],
                            start=True, stop=True)
                        nc.vector.tensor_copy(er_sh[:m, wi, :h2], ps[:m, :h2])
                    nc.gpsimd.collective_compute(
                        "AllGather", mybir.AluOpType.bypass, rg,
                        ins=[cc_in[2][:, :]], outs=[T[2][:, :]])
                elif li < 5:
                    fin2, fo2, h2 = LAYERS[li]
                    hf2 = h2 * fo2
                    ru2 = ROWS_U[li]
                    kch = (fin2 + 127) // 128
                    for wi in range(NWIN):
                        m = min(WIN, SH - wi * WIN)
                        ps = pp_z.tile([128, hf2 + 2 * h2], F32, tag="psz")
                        for k in range(kch):
                            kk = min(128, fin2 - k * 128)
                            nc.tensor.matmul(
                                ps[:m, :], lhsT=hT[k][:kk, wi * WIN:wi * WIN + m],
                                rhs=wsb[li][:kk, k, :],
                                start=(k == 0), stop=(k == kch - 1))
                        row_t = zpool.tile([128, ru2], BF16, tag="rowt2")
                        nc.scalar.activation(row_t[:m, :hf2], ps[:m, :hf2], CPY)
                        nc.vector.tensor_copy(
                            row_t[:m, hf2:hf2 + 2 * h2].bitcast(F32),
                            ps[:m, hf2:hf2 + h2])
                        nc.vector.tensor_copy(er_sh[:m, wi, :h2],
                                              ps[:m, hf2 + h2:hf2 + 2 * h2])
                        nc.sync.dma_start(out=cc_in[li][wi * WIN:wi * WIN + m, :],
                                          in_=row_t[:m, :])
                    nc.gpsimd.collective_compute(
                        "AllGather", mybir.AluOpType.bypass, rg,
                        ins=[cc_in[li][:, :]], outs=[T[li][:, :]])
    nc.finalize()
    return nc


_CACHE = {}


def kernel(**inputs):
    import ml_dtypes

    x = np.asarray(inputs["x"], np.float32)
    src = np.asarray(inputs["src"], np.int64)
    dst = np.asarray(inputs["dst"], np.int64)

    calls, win_first, win_last, NCH, idx_streams, ldst_cols, ldst_rows = _prep(src, dst)

    key = (NCH, len(calls))
    if key not in _CACHE:
        _CACHE[key] = _build(calls, win_first, win_last, NCH)
    nc = _CACHE[key]

    bf = ml_dtypes.bfloat16
    common = {"xrows": np.ascontiguousarray(x).astype(bf)}
    for li, (fin, fo, h) in enumerate(LAYERS, 1):
        W = np.asarray(inputs[f"W{li}"], np.float32)
        al = np.asarray(inputs[f"al{li}"], np.float32)
        ar = np.asarray(inputs[f"ar{li}"], np.float32)
        b = np.asarray(inputs[f"b{li}"], np.float32)
        Wr = W.reshape(fin, h, fo)
        wl = np.einsum("ihf,hf->ih", Wr, al)
        wr = np.einsum("ihf,hf->ih", Wr, ar)
        common[f"Waug{li}"] = np.ascontiguousarray(
            np.concatenate([W, wl, wr], axis=1)).astype(bf)
        common[f"bb{li}"] = np.ascontiguousarray(b.reshape(1, -1))

    in_maps = []
    for c in range(NC):
        m = dict(common)
        m["xTs"] = np.ascontiguousarray(x[c * SH:(c + 1) * SH].T).astype(bf)
        m["idxs"] = idx_streams[c]
        m["ldstc"] = ldst_cols[c]
        m["ldstr"] = ldst_rows[c]
        in_maps.append(m)

    from concourse.bass_utils import run_bass_kernel_spmd
    res = run_bass_kernel_spmd(nc, in_maps, core_ids=list(range(NC)))
    global LAST_RESULT
    LAST_RESULT = res
    out = np.concatenate([res.results[c]["out"] for c in range(NC)], axis=0)
    return out.astype(np.float32)


if __name__ == "__main__":
    data = np.load("/tmp/inputs.npz")
    out = kernel(**{k: data[k] for k in data.files})
    exp = np.load("/tmp/expected.npy")
    rel = np.abs(out - exp) / np.abs(exp).max()
    print("rel err:", rel.max(), "mean", rel.mean())


# revision 16
# speedup vs baseline: 1.0652x; 1.0652x over previous
"""5-layer GAT on 8 TRN2 NeuronCores — dst-sharded gather/aggregate kernel (v2).

Strategy (per core c of 8):
  - dst nodes [c*6250, (c+1)*6250) and their (dst-sorted) edges, grouped in
    128-dst windows; window edges split by src half (int16 gather idx limit),
    padded to 128-edge chunks (uniform structure across cores for SPMD).
  - L1: transpose-dma_gather of raw x rows (256B each) + per-chunk PE matmul
    z1 = x_g^T @ [W1|wl1]  (kills the replicated z1 table of v1).
  - L2-5: dma_gather [z|el] rows (bf16) from a replicated table T_l.
  - Scores: el from gather; er expanded per-edge via PE matmul (lhsT=ST mask);
    e = el + er on DVE; LeakyReLU+Exp on the Scalar engine (ACT, alpha=0.2).
  - Aggregation: psw[:, :hf] += S_k^T @ (ex*z), psw[:, hf:hf+h] += S_k^T @ ex
    (two PE matmuls per chunk; no strided DVE appends).
  - Window flush: divide by sum, +bias, elu via ACT Relu/Exp composition,
    transpose h to [feat, node] on PE for the next layer's z matmul.
  - z_{l+1} = h_shard @ [W|wl|wr] per shard; AllGather the packed table.
No tensor_scalar ops (slow path on this DVE); constants via broadcast APs.
"""
import numpy as np

N = 50000
E = 800000
NC = 8
SH = N // NC
WIN = 128
NWIN = (SH + WIN - 1) // WIN
HALF = 25000
LAYERS = [(128, 64, 4), (256, 64, 2), (128, 64, 2), (128, 64, 1), (64, 4, 1)]
NEG = 0.2
MAXCH = 8


def _row_units(hf, h):
    u = hf + 2 * h
    return ((u + 127) // 128) * 128


ROWS_U = [_row_units(h * f, h) for (_, f, h) in LAYERS]


def _prep(src, dst):
    order = np.argsort(dst, kind="stable")
    src_s, dst_s = src[order], dst[order]
    core_of = dst_s // SH
    core_lists = []
    nch = np.zeros((NWIN, 2), np.int64)
    for c in range(NC):
        m = core_of == c
        s, d = src_s[m], dst_s[m] - c * SH
        w = d // WIN
        lists = {}
        for wi in range(NWIN):
            mw = w == wi
            sw, dw = s[mw], d[mw]
            lo = sw < HALF
            lists[(wi, 0)] = (sw[lo].astype(np.int64), dw[lo])
            lists[(wi, 1)] = (sw[~lo].astype(np.int64) - HALF, dw[~lo])
            for hf in range(2):
                nch[wi, hf] = max(nch[wi, hf],
                                  (len(lists[(wi, hf)][0]) + 127) // 128)
        core_lists.append(lists)
    nch = np.maximum(nch, 1)

    calls = []            # (win, half, c0, g)
    win_first = {}
    win_last = {}
    c0 = 0
    for wi in range(NWIN):
        win_first[wi] = c0
        for hf in range(2):
            n = int(nch[wi, hf])
            k = 0
            while k < n:
                g = min(MAXCH, n - k)
                calls.append((wi, hf, c0 + k, g))
                k += g
            c0 += n
        win_last[wi] = c0 - 1
    NCH = c0
    EPAD = NCH * 128

    idx_streams, ldst_cols, ldst_rows = [], [], []
    for c in range(NC):
        lists = core_lists[c]
        idx = np.zeros(EPAD, np.int64)
        ld = np.full(EPAD, -1, np.int64)
        pos = 0
        for wi in range(NWIN):
            for hf in range(2):
                s, d = lists[(wi, hf)]
                n = int(nch[wi, hf]) * 128
                idx[pos:pos + len(s)] = s
                ld[pos:pos + len(d)] = d % WIN
                pos += n
        blocks = []
        for (wi, hf, cc0, g) in calls:
            blk = idx[cc0 * 128:(cc0 + g) * 128].astype(np.int16)
            blocks.append(np.tile(blk.reshape(-1, 16).T, (8, 1)))
        idx_streams.append(np.ascontiguousarray(np.concatenate(blocks, axis=1)))
        ldst_cols.append(np.ascontiguousarray(
            ld.reshape(NCH, 128).T.astype(np.int8)))
        rows = np.zeros((len(calls), 1, MAXCH * 128), np.int8)
        for i, (wi, hf, cc0, g) in enumerate(calls):
            rows[i, 0, :g * 128] = ld[cc0 * 128:(cc0 + g) * 128]
        ldst_rows.append(rows)

    return calls, win_first, win_last, NCH, idx_streams, ldst_cols, ldst_rows


def _build(calls, win_first, win_last, NCH, only=None):
    from contextlib import ExitStack
    import concourse.bass as bass
    import concourse.bacc as bacc
    import concourse.tile as tile
    from concourse import mybir
    from concourse.masks import make_identity

    F32, BF16, I16 = mybir.dt.float32, mybir.dt.bfloat16, mybir.dt.int16
    I8 = mybir.dt.int8
    EXP = mybir.ActivationFunctionType.Exp
    CPY = mybir.ActivationFunctionType.Copy
    RELU = mybir.ActivationFunctionType.Relu
    EQ = mybir.AluOpType.is_equal
    MAX = mybir.AluOpType.max
    MUL = mybir.AluOpType.mult
    ADD = mybir.AluOpType.add
    TOT16 = NCH * 8
    NCALLS = len(calls)

    nc = bacc.Bacc("TRN2", num_devices=NC, num_swdge_queues=4)

    xrow_d = nc.dram_tensor("xrows", [N, 128], BF16, kind="ExternalInput")
    xTs = nc.dram_tensor("xTs", [128, SH], BF16, kind="ExternalInput")
    Waug, btens = [], []
    for li, (fin, fo, h) in enumerate(LAYERS, 1):
        hf = h * fo
        Waug.append(nc.dram_tensor(f"Waug{li}", [fin, hf + 2 * h], BF16,
                                   kind="ExternalInput"))
        btens.append(nc.dram_tensor(f"bb{li}", [1, hf], F32, kind="ExternalInput"))
    idxs_d = nc.dram_tensor("idxs", [128, TOT16], I16, kind="ExternalInput")
    ldstc_d = nc.dram_tensor("ldstc", [128, NCH], I8, kind="ExternalInput")
    ldstr_d = nc.dram_tensor("ldstr", [NCALLS, 1, MAXCH * 128], I8,
                             kind="ExternalInput")
    out_d = nc.dram_tensor("out", [SH, 4], F32, kind="ExternalOutput")

    T = [None]  # layer-1 gathers straight from xrows
    cc_in = [None]
    for li in range(2, 6):
        u = ROWS_U[li - 1]
        cc_in.append(nc.dram_tensor(f"ccin{li}", [SH, u], BF16, kind="Internal"))
        T.append(nc.dram_tensor(f"T{li}", [N, u], BF16, kind="Internal",
                                addr_space="Shared"))
    rg = [list(range(NC))]

    def bcast_last(ap, n):
        """free-broadcast an AP whose last dim is [*, 1] to [0, n]."""
        a = [list(p) for p in ap.ap]
        assert a[-1][1] == 1
        a[-1] = [0, n]
        return bass.AP(tensor=ap.tensor, offset=ap.offset, ap=a)

    def ap3(base, dims):
        """3D AP on a 2D slice: dims = [[s1,n1],[s2,n2]] appended to partition dim."""
        return bass.AP(tensor=base.tensor, offset=base.offset,
                       ap=[list(base.ap[0])] + [list(d) for d in dims])

    with tile.TileContext(nc) as tc:
        with ExitStack() as ctx:
            cpool = ctx.enter_context(tc.tile_pool(name="const", bufs=1))
            gpool = ctx.enter_context(tc.tile_pool(name="gat", bufs=10))
            xgpool = ctx.enter_context(tc.tile_pool(name="xg", bufs=4))
            spool = ctx.enter_context(tc.tile_pool(name="masks", bufs=6))
            rpool = ctx.enter_context(tc.tile_pool(name="rhs", bufs=4))
            zxpool = ctx.enter_context(tc.tile_pool(name="zx", bufs=3))
            epool = ctx.enter_context(tc.tile_pool(name="expx", bufs=8))
            lpool = ctx.enter_context(tc.tile_pool(name="ldr", bufs=6))
            wpool = ctx.enter_context(tc.tile_pool(name="wflush", bufs=4))
            zpool = ctx.enter_context(tc.tile_pool(name="zphase", bufs=3))
            pp_w = ctx.enter_context(tc.tile_pool(name="ps_w", bufs=2, space="PSUM"))
            pp_er = ctx.enter_context(tc.tile_pool(name="ps_er", bufs=2, space="PSUM"))
            pp_z = ctx.enter_context(tc.tile_pool(name="ps_z", bufs=2, space="PSUM"))
            pp_s = ctx.enter_context(tc.tile_pool(name="ps_s", bufs=2, space="PSUM"))

            iota_row = cpool.tile([128, 128], I8)
            nc.gpsimd.iota(iota_row[:, :], pattern=[[1, 128]], base=0,
                           channel_multiplier=0,
                           allow_small_or_imprecise_dtypes=True)
            iota_col = cpool.tile([128, 128], I8)
            nc.gpsimd.iota(iota_col[:, :], pattern=[[0, 128]], base=0,
                           channel_multiplier=1,
                           allow_small_or_imprecise_dtypes=True)
            ident = cpool.tile([128, 128], BF16)
            make_identity(nc, ident[:, :])
            konst = cpool.tile([128, 2], F32)
            nc.vector.memset(konst[:, 0:1], 1e-30)
            nc.vector.memset(konst[:, 1:2], -1.0)

            idx_sb = cpool.tile([128, TOT16], I16)
            nc.sync.dma_start(out=idx_sb[:, :], in_=idxs_d[:, :])
            ldstc_sb = cpool.tile([128, NCH], I8)
            nc.sync.dma_start(out=ldstc_sb[:, :], in_=ldstc_d[:, :])

            wsb, bsb = [], []
            for li, (fin, fo, h) in enumerate(LAYERS, 1):
                hf = h * fo
                cols = hf + 2 * h
                kch = (fin + 127) // 128
                wt = cpool.tile([128, kch, cols], BF16, tag=f"w{li}")
                if kch > 1:
                    nc.sync.dma_start(
                        out=wt[:, :, :],
                        in_=Waug[li - 1][:, :].rearrange("(k p) c -> p k c", p=128))
                else:
                    nc.sync.dma_start(out=wt[:fin, 0, :], in_=Waug[li - 1][:, :])
                wsb.append(wt)
                bt = cpool.tile([128, hf], F32, tag=f"b{li}")
                bsrc = btens[li - 1][:, :]
                nc.sync.dma_start(out=bt[:, :], in_=bass.AP(
                    tensor=bsrc.tensor, offset=bsrc.offset,
                    ap=[[0, 128]] + [list(p) for p in bsrc.ap[1:]]))
                bsb.append(bt)

            er_sh = cpool.tile([128, NWIN, 4], BF16)
            hT = [cpool.tile([128, SH], BF16, tag=f"hT{k}", name=f"hT{k}")
                  for k in range(2)]

            # ---------- L1 er for own shard (from xTs) ----------
            fin, fo, h = LAYERS[0]
            hf = h * fo
            xts_sb = cpool.tile([128, SH], BF16)
            nc.sync.dma_start(out=xts_sb[:, :], in_=xTs[:, :])
            for wi in range(NWIN):
                m = min(WIN, SH - wi * WIN)
                ps = pp_er.tile([128, MAXCH * 4], F32, tag="erps")
                nc.tensor.matmul(ps[:m, :h], lhsT=xts_sb[:, wi * WIN:wi * WIN + m],
                                 rhs=wsb[0][:, 0, hf + h:hf + 2 * h],
                                 start=True, stop=True)
                nc.vector.tensor_copy(er_sh[:m, wi, :h], ps[:m, :h])

            # ---------- layers ----------
            for li, (fin, fo, h) in enumerate(LAYERS, 1):
                if only == "l1" and li > 1:
                    break
                hf = h * fo
                ru = ROWS_U[li - 1]
                psw = None
                off16 = 0
                if only == "l25" and li == 1:
                    continue
                for ci, (wi, half, c0, g) in enumerate(calls):
                    ni = g * 128
                    if c0 == win_first[wi]:
                        psw = pp_w.tile([128, hf + h], F32, tag="psw")
                        psums = pp_s.tile([128, 4], F32, tag="psums")

                    if li == 1:
                        ftbl = xrow_d
                        base = (ftbl[0:HALF, :] if half == 0
                                else ftbl[HALF:N, :])
                        xg = xgpool.tile([128, 1, MAXCH * 128], BF16, tag="xg")
                        # transpose-mode SWDGE gathers wedge beyond 512 idxs
                        for s0 in range(0, ni, 512):
                            nn = min(512, ni - s0)
                            nc.gpsimd.dma_gather(
                                xg[:, :, s0:s0 + nn], base,
                                idx_sb[:, off16 + s0 // 16:
                                       off16 + (s0 + nn) // 16],
                                num_idxs=nn, num_idxs_reg=nn, elem_size=128,
                                transpose=True,
                                queue_num=(ci + s0 // 512) % 4)
                    else:
                        tbl = T[li - 1]
                        base = tbl[0:HALF, :] if half == 0 else tbl[HALF:N, :]
                        g_t = gpool.tile([128, MAXCH, ru], BF16, tag="gt")
                        nc.gpsimd.dma_gather(
                            g_t[:, :g, :], base, idx_sb[:, off16:off16 + g * 8],
                            num_idxs=ni, num_idxs_reg=ni, elem_size=ru,
                            queue_num=ci % 4)
                    off16 += g * 8

                    ldr = lpool.tile([128, MAXCH * 128], I8)
                    lsrc = ldstr_d[ci, :, :g * 128]
                    nc.sync.dma_start(out=ldr[:, :g * 128], in_=bass.AP(
                        tensor=lsrc.tensor, offset=lsrc.offset,
                        ap=[[0, 128]] + [list(p) for p in lsrc.ap[1:]]))

                    S_b = spool.tile([128, MAXCH, 128], BF16, tag="S")
                    in0 = bass.AP(tensor=ldstc_sb[:, :].tensor,
                                  offset=ldstc_sb[:, :].offset + c0,
                                  ap=[[NCH, 128], [1, g], [0, 128]])
                    in1 = bass.AP(tensor=iota_row[:, :].tensor,
                                  offset=iota_row[:, :].offset,
                                  ap=[[128, 128], [0, g], [1, 128]])
                    nc.vector.tensor_tensor(out=S_b[:, :g, :], in0=in0, in1=in1, op=EQ)

                    ST_b = spool.tile([128, MAXCH, 128], BF16, tag="ST")
                    in0 = bass.AP(tensor=iota_col[:, :].tensor,
                                  offset=iota_col[:, :].offset,
                                  ap=[[128, 128], [0, g], [1, 128]])
                    in1 = bass.AP(tensor=ldr[:, :].tensor,
                                  offset=ldr[:, :].offset,
                                  ap=[[MAXCH * 128, 128], [128, g], [1, 128]])
                    nc.vector.tensor_tensor(out=ST_b[:, :g, :], in0=in0, in1=in1, op=EQ)

                    er_ps = pp_er.tile([128, MAXCH * 4], F32, tag="erps")
                    for k in range(g):
                        nc.tensor.matmul(er_ps[:, k * h:(k + 1) * h],
                                         lhsT=ST_b[:, k, :],
                                         rhs=er_sh[:, wi, :h], start=True, stop=True)

                    if li == 1:
                        er_s = epool.tile([128, MAXCH * 4], F32, tag="ers")
                        nc.vector.tensor_copy(er_s[:, :g * h], er_ps[:, :g * h])
                        for k in range(g):
                            kg = c0 + k
                            psz = pp_z.tile([128, hf + h], F32, tag="psz")
                            nc.tensor.matmul(
                                psz[:, :], lhsT=xg[:, 0, k * 128:(k + 1) * 128],
                                rhs=wsb[0][:, 0, :hf + h], start=True, stop=True)
                            e1 = epool.tile([128, 4], F32, tag="e1")
                            nc.vector.tensor_tensor(
                                out=e1[:, :h], in0=psz[:, hf:hf + h],
                                in1=er_s[:, k * h:(k + 1) * h], op=ADD)
                            q1 = epool.tile([128, 4], F32, tag="q1")
                            nc.scalar.activation(q1[:, :h], e1[:, :h], EXP)
                            q2 = epool.tile([128, 4], F32, tag="q2")
                            nc.scalar.activation(q2[:, :h], e1[:, :h], EXP,
                                                 scale=NEG)
                            xb = epool.tile([128, 4], BF16, tag="xb")
                            nc.vector.tensor_tensor(out=xb[:, :h], in0=q1[:, :h],
                                                    in1=q2[:, :h], op=MAX)
                            rhs1 = rpool.tile([128, hf], BF16, tag="rhs1")
                            nc.vector.tensor_tensor(
                                out=ap3(rhs1[:, :], [[fo, h], [1, fo]]),
                                in0=ap3(psz[:, :], [[fo, h], [1, fo]]),
                                in1=ap3(xb[:, :], [[1, h], [0, fo]]), op=MUL)
                            nc.tensor.matmul(psw[:, :hf], lhsT=S_b[:, k, :],
                                             rhs=rhs1[:, :],
                                             start=(kg == win_first[wi]),
                                             stop=(kg == win_last[wi]))
                            nc.tensor.matmul(psums[:, :h], lhsT=S_b[:, k, :],
                                             rhs=xb[:, :h],
                                             start=(kg == win_first[wi]),
                                             stop=(kg == win_last[wi]))
                    else:
                        e_t = epool.tile([128, MAXCH * 4], F32, tag="e")
                        elv = g_t[:, :g, hf:hf + 2 * h].bitcast(F32)
                        erv = bass.AP(tensor=er_ps[:, :].tensor,
                                      offset=er_ps[:, :].offset,
                                      ap=[[MAXCH * 4, 128], [h, g], [1, h]])
                        ev = bass.AP(tensor=e_t[:, :].tensor,
                                     offset=e_t[:, :].offset,
                                     ap=[[MAXCH * 4, 128], [h, g], [1, h]])
                        nc.vector.tensor_tensor(out=ev, in0=elv, in1=erv, op=ADD)
                        q1_t = epool.tile([128, MAXCH * 4], F32, tag="q1t")
                        nc.scalar.activation(q1_t[:, :g * h], e_t[:, :g * h], EXP)
                        q2_t = epool.tile([128, MAXCH * 4], F32, tag="q2t")
                        nc.scalar.activation(q2_t[:, :g * h], e_t[:, :g * h], EXP,
                                             scale=NEG)
                        exb = epool.tile([128, MAXCH * 4], BF16, tag="exb")
                        nc.vector.tensor_tensor(out=exb[:, :g * h],
                                                in0=q1_t[:, :g * h],
                                                in1=q2_t[:, :g * h], op=MAX)

                        rhs_t = rpool.tile([128, MAXCH, hf], BF16, tag="rhs")
                        for hi in range(h):
                            exv = bass.AP(tensor=exb[:, :].tensor,
                                          offset=exb[:, :].offset + hi,
                                          ap=[[MAXCH * 4, 128], [h, g], [0, fo]])
                            nc.vector.tensor_tensor(
                                out=rhs_t[:, :g, hi * fo:(hi + 1) * fo],
                                in0=g_t[:, :g, hi * fo:(hi + 1) * fo],
                                in1=exv, op=MUL)

                        for k in range(g):
                            kg = c0 + k
                            nc.tensor.matmul(psw[:, :hf], lhsT=S_b[:, k, :],
                                             rhs=rhs_t[:, k, :],
                                             start=(kg == win_first[wi]),
                                             stop=(kg == win_last[wi]))
                            nc.tensor.matmul(psums[:, :h], lhsT=S_b[:, k, :],
                                             rhs=exb[:, k * h:(k + 1) * h],
                                             start=(kg == win_first[wi]),
                                             stop=(kg == win_last[wi]))

                    if c0 + g - 1 == win_last[wi]:
                        # -------- window flush --------
                        m = min(WIN, SH - wi * WIN)
                        sg = wpool.tile([128, 4], F32, tag="sg")
                        nc.vector.tensor_tensor(
                            out=sg[:m, :h], in0=psums[:m, :h],
                            in1=bcast_last(konst[:m, 0:1], h), op=MAX)
                        rr = wpool.tile([128, 4], F32, tag="rr")
                        nc.vector.reciprocal(rr[:m, :h], sg[:m, :h])
                        ow = wpool.tile([128, hf], F32, tag="ow")
                        nc.vector.tensor_tensor(
                            out=ap3(ow[:m, :], [[fo, h], [1, fo]]),
                            in0=ap3(psw[:m, :], [[fo, h], [1, fo]]),
                            in1=ap3(rr[:m, :], [[1, h], [0, fo]]), op=MUL)
                        nc.vector.tensor_add(ow[:m, :], ow[:m, :], bsb[li - 1][:m, :])
                        if li == 5:
                            nc.sync.dma_start(out=out_d[wi * WIN:wi * WIN + m, :],
                                              in_=ow[:m, :4])
                        else:
                            p_t = wpool.tile([128, hf], F32, tag="p")
                            nc.scalar.activation(p_t[:m, :], ow[:m, :], RELU)
                            r_t = wpool.tile([128, hf], F32, tag="r")
                            nc.scalar.activation(r_t[:m, :], ow[:m, :], RELU,
                                                 scale=-1.0)
                            q_t = wpool.tile([128, hf], F32, tag="q")
                            nc.scalar.activation(q_t[:m, :], r_t[:m, :], EXP,
                                                 scale=-1.0)
                            s_t = wpool.tile([128, hf], F32, tag="s")
                            nc.vector.tensor_tensor(out=s_t[:m, :], in0=p_t[:m, :],
                                                    in1=q_t[:m, :], op=ADD)
                            hbf = wpool.tile([128, hf], BF16, tag="hbf")
                            nc.vector.tensor_tensor(
                                out=hbf[:m, :], in0=s_t[:m, :],
                                in1=bcast_last(konst[:m, 1:2], hf), op=ADD)
                            for k in range((hf + 127) // 128):
                                kk = min(128, hf - k * 128)
                                pt = pp_er.tile([128, 128], BF16, tag="erps")
                                nc.tensor.transpose(
                                    pt[:kk, :m], hbf[:m, k * 128:k * 128 + kk],
                                    ident[:m, :m])
                                nc.vector.tensor_copy(
                                    hT[k][:kk, wi * WIN:wi * WIN + m], pt[:kk, :m])

                # -------- z phase for next layer + AllGather --------
                if li < 5:
                    fin2, fo2, h2 = LAYERS[li]
                    hf2 = h2 * fo2
                    ru2 = ROWS_U[li]
                    kch = (fin2 + 127) // 128
                    for wi in range(NWIN):
                        m = min(WIN, SH - wi * WIN)
                        ps = pp_z.tile([128, hf2 + 2 * h2], F32, tag="psz")
                        for k in range(kch):
                            kk = min(128, fin2 - k * 128)
                            nc.tensor.matmul(
                                ps[:m, :], lhsT=hT[k][:kk, wi * WIN:wi * WIN + m],
                                rhs=wsb[li][:kk, k, :],
                                start=(k == 0), stop=(k == kch - 1))
                        row_t = zpool.tile([128, ru2], BF16, tag="rowt2")
                        nc.vector.tensor_copy(row_t[:m, :hf2], ps[:m, :hf2])
                        nc.vector.tensor_copy(
                            row_t[:m, hf2:hf2 + 2 * h2].bitcast(F32),
                            ps[:m, hf2:hf2 + h2])
                        nc.vector.tensor_copy(er_sh[:m, wi, :h2],
                                              ps[:m, hf2 + h2:hf2 + 2 * h2])
                        nc.sync.dma_start(out=cc_in[li][wi * WIN:wi * WIN + m, :],
                                          in_=row_t[:m, :])
                    nc.gpsimd.collective_compute(
                        "AllGather", mybir.AluOpType.bypass, rg,
                        ins=[cc_in[li][:, :]], outs=[T[li][:, :]])
    nc.finalize()
    return nc


_CACHE = {}


def kernel(**inputs):
    import ml_dtypes

    x = np.asarray(inputs["x"], np.float32)
    src = np.asarray(inputs["src"], np.int64)
    dst = np.asarray(inputs["dst"], np.int64)

    calls, win_first, win_last, NCH, idx_streams, ldst_cols, ldst_rows = _prep(src, dst)

    key = (NCH, len(calls))
    if key not in _CACHE:
        _CACHE[key] = _build(calls, win_first, win_last, NCH)
    nc = _CACHE[key]

    bf = ml_dtypes.bfloat16
    common = {"xrows": np.ascontiguousarray(x).astype(bf)}
    for li, (fin, fo, h) in enumerate(LAYERS, 1):
        W = np.asarray(inputs[f"W{li}"], np.float32)
        al = np.asarray(inputs[f"al{li}"], np.float32)
        ar = np.asarray(inputs[f"ar{li}"], np.float32)
        b = np.asarray(inputs[f"b{li}"], np.float32)
        Wr = W.reshape(fin, h, fo)
        wl = np.einsum("ihf,hf->ih", Wr, al)
        wr = np.einsum("ihf,hf->ih", Wr, ar)
        common[f"Waug{li}"] = np.ascontiguousarray(
            np.concatenate([W, wl, wr], axis=1)).astype(bf)
        common[f"bb{li}"] = np.ascontiguousarray(b.reshape(1, -1))

    in_maps = []
    for c in range(NC):
        m = dict(common)
        m["xTs"] = np.ascontiguousarray(x[c * SH:(c + 1) * SH].T).astype(bf)
        m["idxs"] = idx_streams[c]
        m["ldstc"] = ldst_cols[c]
        m["ldstr"] = ldst_rows[c]
        in_maps.append(m)

    from concourse.bass_utils import run_bass_kernel_spmd
    res = run_bass_kernel_spmd(nc, in_maps, core_ids=list(range(NC)))
    global LAST_RESULT
    LAST_RESULT = res
    out = np.concatenate([res.results[c]["out"] for c in range(NC)], axis=0)
    return out.astype(np.float32)


if __name__ == "__main__":
    data = np.load("/tmp/inputs.npz")
    out = kernel(**{k: data[k] for k in data.files})
    exp = np.load("/tmp/expected.npy")
    rel = np.abs(out - exp) / np.abs(exp).max()
    print("rel err:", rel.max(), "mean", rel.mean())


# revision 17
# speedup vs baseline: 1.1497x; 1.0793x over previous
"""5-layer GAT on 8 TRN2 NeuronCores — dst-sharded gather/aggregate kernel (v2).

Strategy (per core c of 8):
  - dst nodes [c*6250, (c+1)*6250) and their (dst-sorted) edges, grouped in
    128-dst windows; window edges split by src half (int16 gather idx limit),
    padded to 128-edge chunks (uniform structure across cores for SPMD).
  - L1: transpose-dma_gather of raw x rows (256B each) + per-chunk PE matmul
    z1 = x_g^T @ [W1|wl1]  (kills the replicated z1 table of v1).
  - L2-5: dma_gather [z|el] rows (bf16) from a replicated table T_l.
  - Scores: el from gather; er expanded per-edge via PE matmul (lhsT=ST mask);
    e = el + er on DVE; LeakyReLU+Exp on the Scalar engine (ACT, alpha=0.2).
  - Aggregation: psw[:, :hf] += S_k^T @ (ex*z), psw[:, hf:hf+h] += S_k^T @ ex
    (two PE matmuls per chunk; no strided DVE appends).
  - Window flush: divide by sum, +bias, elu via ACT Relu/Exp composition,
    transpose h to [feat, node] on PE for the next layer's z matmul.
  - z_{l+1} = h_shard @ [W|wl|wr] per shard; AllGather the packed table.
No tensor_scalar ops (slow path on this DVE); constants via broadcast APs.
"""
import numpy as np

N = 50000
E = 800000
NC = 8
SH = N // NC
WIN = 128
NWIN = (SH + WIN - 1) // WIN
HALF = 25000
LAYERS = [(128, 64, 4), (256, 64, 2), (128, 64, 2), (128, 64, 1), (64, 4, 1)]
NEG = 0.2
MAXCH = 8


def _row_units(hf, h):
    u = hf + 2 * h
    return ((u + 127) // 128) * 128


ROWS_U = [_row_units(h * f, h) for (_, f, h) in LAYERS]


def _prep(src, dst):
    order = np.argsort(dst, kind="stable")
    src_s, dst_s = src[order], dst[order]
    core_of = dst_s // SH
    core_lists = []
    nch = np.zeros((NWIN, 2), np.int64)
    for c in range(NC):
        m = core_of == c
        s, d = src_s[m], dst_s[m] - c * SH
        w = d // WIN
        lists = {}
        for wi in range(NWIN):
            mw = w == wi
            sw, dw = s[mw], d[mw]
            lo = sw < HALF
            lists[(wi, 0)] = (sw[lo].astype(np.int64), dw[lo])
            lists[(wi, 1)] = (sw[~lo].astype(np.int64) - HALF, dw[~lo])
            for hf in range(2):
                nch[wi, hf] = max(nch[wi, hf],
                                  (len(lists[(wi, hf)][0]) + 127) // 128)
        core_lists.append(lists)
    nch = np.maximum(nch, 1)

    calls = []            # (win, half, c0, g)
    win_first = {}
    win_last = {}
    c0 = 0
    for wi in range(NWIN):
        win_first[wi] = c0
        for hf in range(2):
            n = int(nch[wi, hf])
            k = 0
            while k < n:
                g = min(MAXCH, n - k)
                calls.append((wi, hf, c0 + k, g))
                k += g
            c0 += n
        win_last[wi] = c0 - 1
    NCH = c0
    EPAD = NCH * 128

    idx_streams, ldst_cols, ldst_rows = [], [], []
    for c in range(NC):
        lists = core_lists[c]
        idx = np.zeros(EPAD, np.int64)
        ld = np.full(EPAD, -1, np.int64)
        pos = 0
        for wi in range(NWIN):
            for hf in range(2):
                s, d = lists[(wi, hf)]
                n = int(nch[wi, hf]) * 128
                idx[pos:pos + len(s)] = s
                ld[pos:pos + len(d)] = d % WIN
                pos += n
        blocks = []
        for (wi, hf, cc0, g) in calls:
            blk = idx[cc0 * 128:(cc0 + g) * 128].astype(np.int16)
            blocks.append(np.tile(blk.reshape(-1, 16).T, (8, 1)))
        idx_streams.append(np.ascontiguousarray(np.concatenate(blocks, axis=1)))
        ldst_cols.append(np.ascontiguousarray(
            ld.reshape(NCH, 128).T.astype(np.int16)))
        rows = np.zeros((len(calls), 1, MAXCH * 128), np.int16)
        for i, (wi, hf, cc0, g) in enumerate(calls):
            rows[i, 0, :g * 128] = ld[cc0 * 128:(cc0 + g) * 128]
        ldst_rows.append(rows)

    return calls, win_first, win_last, NCH, idx_streams, ldst_cols, ldst_rows


def _build(calls, win_first, win_last, NCH, only=None):
    from contextlib import ExitStack
    import concourse.bass as bass
    import concourse.bacc as bacc
    import concourse.tile as tile
    from concourse import mybir
    from concourse.masks import make_identity

    F32, BF16, I16 = mybir.dt.float32, mybir.dt.bfloat16, mybir.dt.int16
    I8 = mybir.dt.int8
    EXP = mybir.ActivationFunctionType.Exp
    CPY = mybir.ActivationFunctionType.Copy
    RELU = mybir.ActivationFunctionType.Relu
    EQ = mybir.AluOpType.is_equal
    MAX = mybir.AluOpType.max
    MUL = mybir.AluOpType.mult
    ADD = mybir.AluOpType.add
    TOT16 = NCH * 8
    NCALLS = len(calls)

    nc = bacc.Bacc("TRN2", num_devices=NC, num_swdge_queues=4)

    xrow_d = nc.dram_tensor("xrows", [N, 128], BF16, kind="ExternalInput")
    xTs = nc.dram_tensor("xTs", [128, SH], BF16, kind="ExternalInput")
    Waug, btens = [], []
    for li, (fin, fo, h) in enumerate(LAYERS, 1):
        hf = h * fo
        Waug.append(nc.dram_tensor(f"Waug{li}", [fin, hf + 2 * h], BF16,
                                   kind="ExternalInput"))
        btens.append(nc.dram_tensor(f"bb{li}", [1, hf], F32, kind="ExternalInput"))
    idxs_d = nc.dram_tensor("idxs", [128, TOT16], I16, kind="ExternalInput")
    ldstc_d = nc.dram_tensor("ldstc", [128, NCH], I16, kind="ExternalInput")
    ldstr_d = nc.dram_tensor("ldstr", [NCALLS, 1, MAXCH * 128], I16,
                             kind="ExternalInput")
    out_d = nc.dram_tensor("out", [SH, 4], F32, kind="ExternalOutput")

    T = [None]  # layer-1 gathers straight from xrows
    cc_in = [None]
    for li in range(2, 6):
        u = ROWS_U[li - 1]
        cc_in.append(nc.dram_tensor(f"ccin{li}", [SH, u], BF16, kind="Internal"))
        T.append(nc.dram_tensor(f"T{li}", [N, u], BF16, kind="Internal",
                                addr_space="Shared"))
    rg = [list(range(NC))]

    def bcast_last(ap, n):
        """free-broadcast an AP whose last dim is [*, 1] to [0, n]."""
        a = [list(p) for p in ap.ap]
        assert a[-1][1] == 1
        a[-1] = [0, n]
        return bass.AP(tensor=ap.tensor, offset=ap.offset, ap=a)

    def ap3(base, dims):
        """3D AP on a 2D slice: dims = [[s1,n1],[s2,n2]] appended to partition dim."""
        return bass.AP(tensor=base.tensor, offset=base.offset,
                       ap=[list(base.ap[0])] + [list(d) for d in dims])

    with tile.TileContext(nc) as tc:
        with ExitStack() as ctx:
            cpool = ctx.enter_context(tc.tile_pool(name="const", bufs=1))
            gpool = ctx.enter_context(tc.tile_pool(name="gat", bufs=10))
            xgpool = ctx.enter_context(tc.tile_pool(name="xg", bufs=4))
            spool = ctx.enter_context(tc.tile_pool(name="masks", bufs=6))
            rpool = ctx.enter_context(tc.tile_pool(name="rhs", bufs=4))
            zxpool = ctx.enter_context(tc.tile_pool(name="zx", bufs=3))
            epool = ctx.enter_context(tc.tile_pool(name="expx", bufs=8))
            lpool = ctx.enter_context(tc.tile_pool(name="ldr", bufs=6))
            wpool = ctx.enter_context(tc.tile_pool(name="wflush", bufs=4))
            zpool = ctx.enter_context(tc.tile_pool(name="zphase", bufs=3))
            pp_w = ctx.enter_context(tc.tile_pool(name="ps_w", bufs=2, space="PSUM"))
            pp_er = ctx.enter_context(tc.tile_pool(name="ps_er", bufs=2, space="PSUM"))
            pp_z = ctx.enter_context(tc.tile_pool(name="ps_z", bufs=2, space="PSUM"))
            pp_s = ctx.enter_context(tc.tile_pool(name="ps_s", bufs=2, space="PSUM"))

            iota_row = cpool.tile([128, 128], I16)
            nc.gpsimd.iota(iota_row[:, :], pattern=[[1, 128]], base=0,
                           channel_multiplier=0)
            iota_col = cpool.tile([128, 128], I16)
            nc.gpsimd.iota(iota_col[:, :], pattern=[[0, 128]], base=0,
                           channel_multiplier=1)
            ident = cpool.tile([128, 128], BF16)
            make_identity(nc, ident[:, :])
            konst = cpool.tile([128, 2], F32)
            nc.vector.memset(konst[:, 0:1], 1e-30)
            nc.vector.memset(konst[:, 1:2], -1.0)

            idx_sb = cpool.tile([128, TOT16], I16)
            nc.sync.dma_start(out=idx_sb[:, :], in_=idxs_d[:, :])
            ldstc_sb = cpool.tile([128, NCH], I16)
            nc.sync.dma_start(out=ldstc_sb[:, :], in_=ldstc_d[:, :])

            wsb, bsb = [], []
            for li, (fin, fo, h) in enumerate(LAYERS, 1):
                hf = h * fo
                cols = hf + 2 * h
                kch = (fin + 127) // 128
                wt = cpool.tile([128, kch, cols], BF16, tag=f"w{li}")
                if kch > 1:
                    nc.sync.dma_start(
                        out=wt[:, :, :],
                        in_=Waug[li - 1][:, :].rearrange("(k p) c -> p k c", p=128))
                else:
                    nc.sync.dma_start(out=wt[:fin, 0, :], in_=Waug[li - 1][:, :])
                wsb.append(wt)
                bt = cpool.tile([128, hf], F32, tag=f"b{li}")
                bsrc = btens[li - 1][:, :]
                nc.sync.dma_start(out=bt[:, :], in_=bass.AP(
                    tensor=bsrc.tensor, offset=bsrc.offset,
                    ap=[[0, 128]] + [list(p) for p in bsrc.ap[1:]]))
                bsb.append(bt)

            er_sh = cpool.tile([128, NWIN, 4], BF16)
            hT = [cpool.tile([128, SH], BF16, tag=f"hT{k}", name=f"hT{k}")
                  for k in range(2)]

            # ---------- L1 er for own shard (from xTs) ----------
            fin, fo, h = LAYERS[0]
            hf = h * fo
            xts_sb = cpool.tile([128, SH], BF16)
            nc.sync.dma_start(out=xts_sb[:, :], in_=xTs[:, :])
            for wi in range(NWIN):
                m = min(WIN, SH - wi * WIN)
                ps = pp_er.tile([128, MAXCH * 4], F32, tag="erps")
                nc.tensor.matmul(ps[:m, :h], lhsT=xts_sb[:, wi * WIN:wi * WIN + m],
                                 rhs=wsb[0][:, 0, hf + h:hf + 2 * h],
                                 start=True, stop=True)
                nc.vector.tensor_copy(er_sh[:m, wi, :h], ps[:m, :h])

            # ---------- layers ----------
            for li, (fin, fo, h) in enumerate(LAYERS, 1):
                if only == "l1" and li > 1:
                    break
                hf = h * fo
                ru = ROWS_U[li - 1]
                psw = None
                off16 = 0
                if only == "l25" and li == 1:
                    continue
                for ci, (wi, half, c0, g) in enumerate(calls):
                    ni = g * 128
                    if c0 == win_first[wi]:
                        psw = pp_w.tile([128, hf + h], F32, tag="psw")
                        psums = pp_s.tile([128, 4], F32, tag="psums")

                    if li == 1:
                        ftbl = xrow_d
                        base = (ftbl[0:HALF, :] if half == 0
                                else ftbl[HALF:N, :])
                        xg = xgpool.tile([128, 1, MAXCH * 128], BF16, tag="xg")
                        # transpose-mode SWDGE gathers wedge beyond 512 idxs
                        for s0 in range(0, ni, 512):
                            nn = min(512, ni - s0)
                            nc.gpsimd.dma_gather(
                                xg[:, :, s0:s0 + nn], base,
                                idx_sb[:, off16 + s0 // 16:
                                       off16 + (s0 + nn) // 16],
                                num_idxs=nn, num_idxs_reg=nn, elem_size=128,
                                transpose=True,
                                queue_num=(ci + s0 // 512) % 4)
                    else:
                        tbl = T[li - 1]
                        base = tbl[0:HALF, :] if half == 0 else tbl[HALF:N, :]
                        g_t = gpool.tile([128, MAXCH, ru], BF16, tag="gt")
                        nc.gpsimd.dma_gather(
                            g_t[:, :g, :], base, idx_sb[:, off16:off16 + g * 8],
                            num_idxs=ni, num_idxs_reg=ni, elem_size=ru,
                            queue_num=ci % 4)
                    off16 += g * 8

                    ldr = lpool.tile([128, MAXCH * 128], I16)
                    lsrc = ldstr_d[ci, :, :g * 128]
                    nc.sync.dma_start(out=ldr[:, :g * 128], in_=bass.AP(
                        tensor=lsrc.tensor, offset=lsrc.offset,
                        ap=[[0, 128]] + [list(p) for p in lsrc.ap[1:]]))

                    S_b = spool.tile([128, MAXCH, 128], BF16, tag="S")
                    in0 = bass.AP(tensor=ldstc_sb[:, :].tensor,
                                  offset=ldstc_sb[:, :].offset + c0,
                                  ap=[[NCH, 128], [1, g], [0, 128]])
                    in1 = bass.AP(tensor=iota_row[:, :].tensor,
                                  offset=iota_row[:, :].offset,
                                  ap=[[128, 128], [0, g], [1, 128]])
                    nc.vector.tensor_tensor(out=S_b[:, :g, :], in0=in0, in1=in1, op=EQ)

                    ST_b = spool.tile([128, MAXCH, 128], BF16, tag="ST")
                    in0 = bass.AP(tensor=iota_col[:, :].tensor,
                                  offset=iota_col[:, :].offset,
                                  ap=[[128, 128], [0, g], [1, 128]])
                    in1 = bass.AP(tensor=ldr[:, :].tensor,
                                  offset=ldr[:, :].offset,
                                  ap=[[MAXCH * 128, 128], [128, g], [1, 128]])
                    nc.vector.tensor_tensor(out=ST_b[:, :g, :], in0=in0, in1=in1, op=EQ)

                    er_ps = pp_er.tile([128, MAXCH * 4], F32, tag="erps")
                    for k in range(g):
                        nc.tensor.matmul(er_ps[:, k * h:(k + 1) * h],
                                         lhsT=ST_b[:, k, :],
                                         rhs=er_sh[:, wi, :h], start=True, stop=True)

                    if li == 1:
                        er_s = epool.tile([128, MAXCH * 4], F32, tag="ers")
                        nc.vector.tensor_copy(er_s[:, :g * h], er_ps[:, :g * h])
                        for k in range(g):
                            kg = c0 + k
                            psz = pp_z.tile([128, hf + h], F32, tag="psz")
                            nc.tensor.matmul(
                                psz[:, :], lhsT=xg[:, 0, k * 128:(k + 1) * 128],
                                rhs=wsb[0][:, 0, :hf + h], start=True, stop=True)
                            e1 = epool.tile([128, 4], F32, tag="e1")
                            nc.vector.tensor_tensor(
                                out=e1[:, :h], in0=psz[:, hf:hf + h],
                                in1=er_s[:, k * h:(k + 1) * h], op=ADD)
                            q1 = epool.tile([128, 4], F32, tag="q1")
                            nc.scalar.activation(q1[:, :h], e1[:, :h], EXP)
                            q2 = epool.tile([128, 4], F32, tag="q2")
                            nc.scalar.activation(q2[:, :h], e1[:, :h], EXP,
                                                 scale=NEG)
                            xb = epool.tile([128, 4], BF16, tag="xb")
                            nc.vector.tensor_tensor(out=xb[:, :h], in0=q1[:, :h],
                                                    in1=q2[:, :h], op=MAX)
                            rhs1 = rpool.tile([128, hf], BF16, tag="rhs1")
                            nc.vector.tensor_tensor(
                                out=ap3(rhs1[:, :], [[fo, h], [1, fo]]),
                                in0=ap3(psz[:, :], [[fo, h], [1, fo]]),
                                in1=ap3(xb[:, :], [[1, h], [0, fo]]), op=MUL)
                            nc.tensor.matmul(psw[:, :hf], lhsT=S_b[:, k, :],
                                             rhs=rhs1[:, :],
                                             start=(kg == win_first[wi]),
                                             stop=(kg == win_last[wi]))
                            nc.tensor.matmul(psums[:, :h], lhsT=S_b[:, k, :],
                                             rhs=xb[:, :h],
                                             start=(kg == win_first[wi]),
                                             stop=(kg == win_last[wi]))
                    else:
                        e_t = epool.tile([128, MAXCH * 4], F32, tag="e")
                        elv = g_t[:, :g, hf:hf + 2 * h].bitcast(F32)
                        erv = bass.AP(tensor=er_ps[:, :].tensor,
                                      offset=er_ps[:, :].offset,
                                      ap=[[MAXCH * 4, 128], [h, g], [1, h]])
                        ev = bass.AP(tensor=e_t[:, :].tensor,
                                     offset=e_t[:, :].offset,
                                     ap=[[MAXCH * 4, 128], [h, g], [1, h]])
                        nc.vector.tensor_tensor(out=ev, in0=elv, in1=erv, op=ADD)
                        q1_t = epool.tile([128, MAXCH * 4], F32, tag="q1t")
                        nc.scalar.activation(q1_t[:, :g * h], e_t[:, :g * h], EXP)
                        q2_t = epool.tile([128, MAXCH * 4], F32, tag="q2t")
                        nc.scalar.activation(q2_t[:, :g * h], e_t[:, :g * h], EXP,
                                             scale=NEG)
                        exb = epool.tile([128, MAXCH * 4], BF16, tag="exb")
                        nc.vector.tensor_tensor(out=exb[:, :g * h],
                                                in0=q1_t[:, :g * h],
                                                in1=q2_t[:, :g * h], op=MAX)

                        rhs_t = rpool.tile([128, MAXCH, hf], BF16, tag="rhs")
                        for hi in range(h):
                            exv = bass.AP(tensor=exb[:, :].tensor,
                                          offset=exb[:, :].offset + hi,
                                          ap=[[MAXCH * 4, 128], [h, g], [0, fo]])
                            nc.vector.tensor_tensor(
                                out=rhs_t[:, :g, hi * fo:(hi + 1) * fo],
                                in0=g_t[:, :g, hi * fo:(hi + 1) * fo],
                                in1=exv, op=MUL)

                        for k in range(g):
                            kg = c0 + k
                            nc.tensor.matmul(psw[:, :hf], lhsT=S_b[:, k, :],
                                             rhs=rhs_t[:, k, :],
                                             start=(kg == win_first[wi]),
                                             stop=(kg == win_last[wi]))
                            nc.tensor.matmul(psums[:, :h], lhsT=S_b[:, k, :],
                                             rhs=exb[:, k * h:(k + 1) * h],
                                             start=(kg == win_first[wi]),
                                             stop=(kg == win_last[wi]))

                    if c0 + g - 1 == win_last[wi]:
                        # -------- window flush --------
                        m = min(WIN, SH - wi * WIN)
                        sg = wpool.tile([128, 4], F32, tag="sg")
                        nc.vector.tensor_tensor(
                            out=sg[:m, :h], in0=psums[:m, :h],
                            in1=bcast_last(konst[:m, 0:1], h), op=MAX)
                        rr = wpool.tile([128, 4], F32, tag="rr")
                        nc.vector.reciprocal(rr[:m, :h], sg[:m, :h])
                        ow = wpool.tile([128, hf], F32, tag="ow")
                        nc.vector.tensor_tensor(
                            out=ap3(ow[:m, :], [[fo, h], [1, fo]]),
                            in0=ap3(psw[:m, :], [[fo, h], [1, fo]]),
                            in1=ap3(rr[:m, :], [[1, h], [0, fo]]), op=MUL)
                        nc.vector.tensor_add(ow[:m, :], ow[:m, :], bsb[li - 1][:m, :])
                        if li == 5:
                            nc.sync.dma_start(out=out_d[wi * WIN:wi * WIN + m, :],
                                              in_=ow[:m, :4])
                        else:
                            p_t = wpool.tile([128, hf], F32, tag="p")
                            nc.scalar.activation(p_t[:m, :], ow[:m, :], RELU)
                            r_t = wpool.tile([128, hf], F32, tag="r")
                            nc.scalar.activation(r_t[:m, :], ow[:m, :], RELU,
                                                 scale=-1.0)
                            q_t = wpool.tile([128, hf], F32, tag="q")
                            nc.scalar.activation(q_t[:m, :], r_t[:m, :], EXP,
                                                 scale=-1.0)
                            s_t = wpool.tile([128, hf], F32, tag="s")
                            nc.vector.tensor_tensor(out=s_t[:m, :], in0=p_t[:m, :],
                                                    in1=q_t[:m, :], op=ADD)
                            hbf = wpool.tile([128, hf], BF16, tag="hbf")
                            nc.vector.tensor_tensor(
                                out=hbf[:m, :], in0=s_t[:m, :],
                                in1=bcast_last(konst[:m, 1:2], hf), op=ADD)
                            for k in range((hf + 127) // 128):
                                kk = min(128, hf - k * 128)
                                pt = pp_er.tile([128, 128], BF16, tag="erps")
                                nc.tensor.transpose(
                                    pt[:kk, :m], hbf[:m, k * 128:k * 128 + kk],
                                    ident[:m, :m])
                                nc.vector.tensor_copy(
                                    hT[k][:kk, wi * WIN:wi * WIN + m], pt[:kk, :m])

                # -------- z phase for next layer + AllGather --------
                if li < 5:
                    fin2, fo2, h2 = LAYERS[li]
                    hf2 = h2 * fo2
                    ru2 = ROWS_U[li]
                    kch = (fin2 + 127) // 128
                    for wi in range(NWIN):
                        m = min(WIN, SH - wi * WIN)
                        ps = pp_z.tile([128, hf2 + 2 * h2], F32, tag="psz")
                        for k in range(kch):
                            kk = min(128, fin2 - k * 128)
                            nc.tensor.matmul(
                                ps[:m, :], lhsT=hT[k][:kk, wi * WIN:wi * WIN + m],
                                rhs=wsb[li][:kk, k, :],
                                start=(k == 0), stop=(k == kch - 1))
                        row_t = zpool.tile([128, ru2], BF16, tag="rowt2")
                        nc.vector.tensor_copy(row_t[:m, :hf2], ps[:m, :hf2])
                        nc.vector.tensor_copy(
                            row_t[:m, hf2:hf2 + 2 * h2].bitcast(F32),
                            ps[:m, hf2:hf2 + h2])
                        nc.vector.tensor_copy(er_sh[:m, wi, :h2],
                                              ps[:m, hf2 + h2:hf2 + 2 * h2])
                        nc.sync.dma_start(out=cc_in[li][wi * WIN:wi * WIN + m, :],
                                          in_=row_t[:m, :])
                    nc.gpsimd.collective_compute(
                        "AllGather", mybir.AluOpType.bypass, rg,
                        ins=[cc_in[li][:, :]], outs=[T[li][:, :]])
    nc.finalize()
    return nc


_CACHE = {}


def kernel(**inputs):
    import ml_dtypes

    x = np.asarray(inputs["x"], np.float32)
    src = np.asarray(inputs["src"], np.int64)
    dst = np.asarray(inputs["dst"], np.int64)

    calls, win_first, win_last, NCH, idx_streams, ldst_cols, ldst_rows = _prep(src, dst)

    key = (NCH, len(calls))
    if key not in _CACHE:
        _CACHE[key] = _build(calls, win_first, win_last, NCH)
    nc = _CACHE[key]

    bf = ml_dtypes.bfloat16
    common = {"xrows": np.ascontiguousarray(x).astype(bf)}
    for li, (fin, fo, h) in enumerate(LAYERS, 1):
        W = np.asarray(inputs[f"W{li}"], np.float32)
        al = np.asarray(inputs[f"al{li}"], np.float32)
        ar = np.asarray(inputs[f"ar{li}"], np.float32)
        b = np.asarray(inputs[f"b{li}"], np.float32)
        Wr = W.reshape(fin, h, fo)
        wl = np.einsum("ihf,hf->ih", Wr, al)
        wr = np.einsum("ihf,hf->ih", Wr, ar)
        common[f"Waug{li}"] = np.ascontiguousarray(
            np.concatenate([W, wl, wr], axis=1)).astype(bf)
        common[f"bb{li}"] = np.ascontiguousarray(b.reshape(1, -1))

    in_maps = []
    for c in range(NC):
        m = dict(common)
        m["xTs"] = np.ascontiguousarray(x[c * SH:(c + 1) * SH].T).astype(bf)
        m["idxs"] = idx_streams[c]
        m["ldstc"] = ldst_cols[c]
        m["ldstr"] = ldst_rows[c]
        in_maps.append(m)

    from concourse.bass_utils import run_bass_kernel_spmd
    res = run_bass_kernel_spmd(nc, in_maps, core_ids=list(range(NC)))
    global LAST_RESULT
    LAST_RESULT = res
    out = np.concatenate([res.results[c]["out"] for c in range(NC)], axis=0)
    return out.astype(np.float32)


if __name__ == "__main__":
    data = np.load("/tmp/inputs.npz")
    out = kernel(**{k: data[k] for k in data.files})
    exp = np.load("/tmp/expected.npy")
    rel = np.abs(out - exp) / np.abs(exp).max()
    print("rel err:", rel.max(), "mean", rel.mean())
